# revision 16
# baseline (speedup 1.0000x reference)
"""W4A16 column-parallel linear kernel for Trainium2 (8 NeuronCores).

y = x @ dequant(qweight_packed, w_scales).T + bias
  x: [4, 2048, 4096] f32
  qweight_packed: [11008, 2048] int32 (two int4 nibbles per byte, low first)
  w_scales: [11008, 1] f32, bias: [11008] f32
  -> y: [4, 2048, 11008] f32

Sharding: column-parallel over out_features (1376 rows of W per core).
Each core keeps its dequantized weight shard resident in SBUF as bf16
(int4 values are exact in bf16), streams x as bf16 [K, M] tiles,
accumulates in f32 PSUM, and applies scale+bias in f32 on PSUM eviction.

K-permutation trick: the matmul contraction is order-invariant, so nibbles
are unpacked to K' = [all low nibbles (k=0,2,4,...), all high nibbles
(k=1,3,5,...)] and x's K axis is permuted to match on the host. This makes
the on-device unpack purely elementwise (no interleave shuffle).
"""

import os
import sys

import numpy as np
import ml_dtypes

for _p in ("/opt/trn_rl_repo", "/root/.axon_site/_ro/trn_rl_repo"):
    if os.path.isdir(_p) and _p not in sys.path:
        sys.path.append(_p)

import concourse.bacc as bacc
import concourse.tile as tile
import concourse.mybir as mybir
from concourse.bass_utils import run_bass_kernel_spmd

dt = mybir.dt
Alu = mybir.AluOpType
BF16 = ml_dtypes.bfloat16

# Problem shape (hardcoded per harness contract)
B, S, K_FULL, N_FULL = 4, 2048, 4096, 11008
N_CORES = 8
M_FULL = B * S            # 8192
KP_FULL = K_FULL // 2     # 2048 packed bytes per W row
N_SH = N_FULL // N_CORES  # 1376
M_SUP = 256               # tokens per x super-tile
P = 128


def build_nc(M, KP, NSH, m_sup=M_SUP, nb_max=512):
    """Build one core's Bass module: y[M, NSH] = x[M, 2KP] @ W[NSH, 2KP].T"""
    K = 2 * KP
    n_kt = K // P
    n_kpt = KP // P
    n_ms = M // m_sup
    n_mi = m_sup // P
    nbs = []
    off = 0
    while off < NSH:
        w = min(nb_max, NSH - off)
        nbs.append((off, w))
        off += w

    nc = bacc.Bacc("TRN2", target_bir_lowering=False, debug=False)
    xt = nc.dram_tensor("xt", [n_ms, n_kt, P, m_sup], dt.bfloat16,
                        kind="ExternalInput")
    qt = nc.dram_tensor("qt", [n_kpt, P, NSH], dt.int8, kind="ExternalInput")
    scb = nc.dram_tensor("scb", [P, NSH], dt.float32, kind="ExternalInput")
    bib = nc.dram_tensor("bib", [P, NSH], dt.float32, kind="ExternalInput")
    y = nc.dram_tensor("y", [M, NSH], dt.float32, kind="ExternalOutput")

    # k consumption order matched to dequant production order
    # (lo_0, hi_0, lo_1, hi_1, ...): [0, n_kpt, 1, n_kpt+1, ...]
    korder = []
    for j in range(n_kpt):
        korder.append(j)
        korder.append(n_kpt + j)

    with tile.TileContext(nc) as tc:
        with (
            tc.tile_pool(name="wpool", bufs=1) as wpool,
            tc.tile_pool(name="qpool", bufs=4) as qpool,
            tc.tile_pool(name="xpool", bufs=2) as xpool,
            tc.tile_pool(name="cpool", bufs=1) as cpool,
            tc.tile_pool(name="opool", bufs=4) as opool,
            tc.tile_pool(name="pspool", bufs=8, space="PSUM") as pspool,
        ):
            sc = cpool.tile([P, NSH], dt.float32, tag="sc")
            nc.gpsimd.dma_start(sc[:], scb[:])
            bi = cpool.tile([P, NSH], dt.float32, tag="bi")
            nc.gpsimd.dma_start(bi[:], bib[:])

            # Dequant W into resident SBUF bf16 tiles (int4 values,
            # unscaled), split across DVE / ACT / GPSIMD so production
            # isn't serialized on one engine.
            m8 = cpool.tile([P, 1], dt.float32, tag="m8")
            nc.vector.memset(m8[:], -8.0)
            LO_MASK = 0x0F0F0F0F
            HI_MASK = -252645136  # 0xF0F0F0F0 as int32
            XOR8 = 0x08080808
            wts = [None] * n_kt
            for j in range(n_kpt):
                u = qpool.tile([P, NSH], dt.int8, tag="q")
                nc.gpsimd.dma_start(u[:], qt[j])
                u32 = u[:].bitcast(dt.int32)
                # low nibble: ((u & 15) ^ 8) - 8; bitwise part done 4
                # bytes at a time in an int32 view, -8+cast on ACT.
                tl = qpool.tile([P, NSH], dt.int8, tag="tl")
                wlo = wpool.tile([P, NSH], dt.bfloat16, tag=f"w{j}")
                nc.vector.tensor_scalar(tl[:].bitcast(dt.int32), u32,
                                        LO_MASK, XOR8,
                                        op0=Alu.bitwise_and,
                                        op1=Alu.bitwise_xor)
                nc.scalar.activation(wlo[:], tl[:],
                                     mybir.ActivationFunctionType.Identity,
                                     bias=m8[:], scale=1.0)
                # high nibble: (u & 0xF0) = 16 * hi (sign included); the
                # 1/16 is folded into the host-side x odd-half scaling.
                th = qpool.tile([P, NSH], dt.int8, tag="th")
                whi = wpool.tile([P, NSH], dt.bfloat16, tag=f"w{n_kpt + j}")
                nc.vector.tensor_scalar(th[:].bitcast(dt.int32), u32,
                                        HI_MASK, None, op0=Alu.bitwise_and)
                nc.vector.tensor_scalar(whi[:], th[:], 0, None, op0=Alu.add)
                wts[j] = wlo
                wts[n_kpt + j] = whi

            groups = [(mi, nb0, nbw) for mi in range(n_mi)
                      for nb0, nbw in nbs]

            def evict(ps, mi, nb0, nbw, ms):
                osb = opool.tile([P, nbw], dt.float32, tag="o")
                nc.vector.tensor_tensor(osb[:], ps[:], sc[:, nb0:nb0 + nbw],
                                        op=Alu.mult)
                nc.vector.tensor_tensor(osb[:], osb[:], bi[:, nb0:nb0 + nbw],
                                        op=Alu.add)
                r0 = ms * m_sup + mi * P
                nc.sync.dma_start(y[r0:r0 + P, nb0:nb0 + nbw], osb[:])

            for ms in range(n_ms):
                xk = []
                for t in range(n_kt):
                    xi = xpool.tile([P, m_sup], dt.bfloat16, tag=f"x{t}")
                    nc.sync.dma_start(xi[:], xt[ms, t])
                    xk.append(xi)
                if ms == 0 and len(groups) <= 8:
                    # k-major across all psum groups: PE consumes each W
                    # tile as dequant produces it instead of stalling on
                    # the full set.
                    pss = []
                    for g, (mi, nb0, nbw) in enumerate(groups):
                        pss.append(pspool.tile([P, nbw], dt.float32,
                                               tag="ps", name=f"ps{g}"))
                    for s, t in enumerate(korder):
                        for g, (mi, nb0, nbw) in enumerate(groups):
                            nc.tensor.matmul(
                                pss[g][:],
                                xk[t][:, mi * P:mi * P + P],
                                wts[t][:, nb0:nb0 + nbw],
                                start=(s == 0),
                                stop=(s == n_kt - 1),
                            )
                    for g, (mi, nb0, nbw) in enumerate(groups):
                        evict(pss[g], mi, nb0, nbw, ms)
                else:
                    for mi, nb0, nbw in groups:
                        ps = pspool.tile([P, nbw], dt.float32, tag="ps")
                        for s, t in enumerate(korder):
                            nc.tensor.matmul(
                                ps[:],
                                xk[t][:, mi * P:mi * P + P],
                                wts[t][:, nb0:nb0 + nbw],
                                start=(s == 0),
                                stop=(s == n_kt - 1),
                            )
                        evict(ps, mi, nb0, nbw, ms)

    nc.compile()
    return nc


def prep_x(x2, m_sup=M_SUP):
    """[M, K] f32 -> [n_ms, n_kt, 128, m_sup] bf16 with K' nibble permutation.

    The odd-k half (matched against high nibbles stored as 16*hi) is
    pre-scaled by 1/16 (exact exponent shift in bf16).
    """
    M, K = x2.shape
    KP = K // 2
    n_ms = M // m_sup
    n_kt = K // P
    xb = x2.reshape(n_ms, m_sup, KP, 2).copy()
    xb[:, :, :, 1] *= np.float32(1.0 / 16.0)
    xb = xb.astype(BF16)
    return np.ascontiguousarray(xb.transpose(0, 3, 2, 1)).reshape(
        n_ms, n_kt, P, m_sup)


def prep_q(q_u8_shard):
    """[NSH, KP] uint8 -> [n_kpt, 128, NSH] int8 (transposed packed bytes)."""
    NSH, KP = q_u8_shard.shape
    return np.ascontiguousarray(q_u8_shard.T).view(np.int8).reshape(
        KP // P, P, NSH)


def prep_bcast(v):
    """[NSH] f32 -> [128, NSH] f32 broadcast tile."""
    return np.ascontiguousarray(
        np.broadcast_to(v.astype(np.float32)[None, :], (P, v.shape[0])))


def _ensure_ntff_hook():
    """Register the axon NTFF profiling hook if the image's antenv lacks
    axon_hooks (trn_boot degrades silently in that case)."""
    try:
        from antenv.axon_hooks import get_axon_ntff_profile_hook  # noqa: F401
        return
    except ImportError:
        pass
    import types
    import antenv
    mod = types.ModuleType("antenv.axon_hooks")
    _h = {"hook": None}
    mod.set_axon_ntff_profile_hook = lambda h: _h.__setitem__("hook", h)
    mod.get_axon_ntff_profile_hook = lambda: _h["hook"]
    sys.modules["antenv.axon_hooks"] = mod
    antenv.axon_hooks = mod
    try:
        from trn_agent_boot.trn_boot import _ntff_profile_via_ctypes
        hook = _ntff_profile_via_ctypes("/opt/axon/libaxon_pjrt.so")
        if hook is not None:
            mod.set_axon_ntff_profile_hook(hook)
    except Exception as e:  # profiling optional; run still works
        print("ntff hook setup failed:", e)


_NC_CACHE = {}


def _get_nc():
    key = (M_FULL, KP_FULL, N_SH, M_SUP)
    if key not in _NC_CACHE:
        _NC_CACHE[key] = build_nc(*key[:3], m_sup=key[3])
    return _NC_CACHE[key]


LAST_RESULT = None


def kernel(x, qweight_packed, w_scales, bias, _profile=False):
    global LAST_RESULT
    x = np.asarray(x)
    qweight_packed = np.asarray(qweight_packed)
    w_scales = np.asarray(w_scales)
    bias = np.asarray(bias)

    if _profile:
        _ensure_ntff_hook()

    nc = _get_nc()

    x2 = np.ascontiguousarray(x.reshape(M_FULL, K_FULL).astype(np.float32))
    xt = prep_x(x2)
    q_u8 = qweight_packed.astype(np.uint8)

    in_maps = []
    for c in range(N_CORES):
        r0, r1 = c * N_SH, (c + 1) * N_SH
        in_maps.append({
            "xt": xt,
            "qt": prep_q(q_u8[r0:r1]),
            "scb": prep_bcast(w_scales[r0:r1, 0]),
            "bib": prep_bcast(bias[r0:r1]),
        })

    res = run_bass_kernel_spmd(nc, in_maps, list(range(N_CORES)),
                               trace=_profile)
    LAST_RESULT = res
    y = np.concatenate([res.results[c]["y"] for c in range(N_CORES)], axis=1)
    return y.reshape(B, S, N_FULL)
